# revision 35
# baseline (speedup 1.0000x reference)
"""AxialSelfAttention2d Trainium2 kernel (8 NeuronCores).

Sharding: stage 1 (row attention, attends along L) is S-sharded (32 rows/core);
stage 2 (column attention, attends along S) is L-sharded (32 cols/core).
Between stages an AllToAll reshards out1 = x + row_out.

Per-core stage structure (identical for both stages; "rows" = s for stage 1,
l for stage 2; "cols" = the 256-long attended axis):
  - QKV 1x1-conv projection as matmuls, q/k in [chan, pix] layout, v projected
    transposed ([pix, chan]) with a ones column appended per head.
  - Per (head, row): attnT[j,i] = k^T q via PE (fp32r), exp on ACT (no max
    subtraction -- logits are bounded ~|10|), AV matmul with the ones row
    yielding unnormalized out + softmax denominators in one pass, reciprocal
    on DVE, K=1 outer-product matmul broadcasts recip over 64 partitions,
    DVE normalize + residual/bias fold (v-bias passes through softmax exactly
    since weights sum to 1).
All matmuls run as float32r (fp32 data, FP22 multiply, fp32 accumulate).
"""

import numpy as np
import concourse.bass as bass
import concourse.tile as tile
import concourse.mybir as mybir
from concourse import bacc
from concourse.bass_utils import run_bass_kernel_spmd

N_CORES = 8
D = 512                 # embed channels
H = 8                   # heads
DH = 64                 # head dim
S = 256
L = 256
RLOC = 32               # rows per core (s-rows stage 1, l-cols stage 2)
PIX = RLOC * 256        # 8192 pixels per core per stage
F32 = mybir.dt.float32
F32R = mybir.dt.float32r
ADD = mybir.AluOpType.add

_CACHE = {}


def _load_weights(nc, sb, prefix, w_ins):
    """DMA weight/bias DRAM inputs into SBUF tiles. Returns dict of tiles."""
    wq_d, wk_d, wv_d, bq_d, bk_d, bv_d = w_ins
    out = {}
    for wname, wd in (("wq", wq_d), ("wk", wk_d), ("wv", wv_d)):
        tiles = []
        for c4 in range(4):
            t = sb.tile([128, 512], F32R, name=f"{prefix}{wname}{c4}", bufs=1)
            nc.sync.dma_start(t[:], wd[c4 * 128:(c4 + 1) * 128, :])
            tiles.append(t)
        out[wname] = tiles
    for bname, bd in (("bq", bq_d), ("bk", bk_d)):
        tiles = []
        for oc in range(4):
            t = sb.tile([128, 1], F32, name=f"{prefix}{bname}{oc}", bufs=1)
            nc.sync.dma_start(t[:], bd[oc * 128:(oc + 1) * 128, :])
            tiles.append(t)
        out[bname] = tiles
    bvr = sb.tile([1, 512], F32R, name=f"{prefix}bvr", bufs=1)
    nc.sync.dma_start(bvr[:],
                      bv_d.rearrange("h d one -> one (h d)").bitcast(F32R))
    out["bvr"] = bvr
    return out


def _stage(tc, nc, sb, ps, w, ones_sb, onescol, bvr, x_load, out_store, prefix,
           stag_dtype=F32):
    """One attention stage over this core's 32 rows.

    Head-pair units: both heads of an o-chunk processed together so the
    normalize/residual chain runs at full 128-partition width.
    x_load(cc, chunk, x_tile): DMA 2-row pixel chunk of x into [128, 512] tile.
    out_store(cc, g, stag_tile): DMA finished [128, 256, 8] staging tile out.
    """
    for g in range(4):                      # groups of 8 rows
        stag = []
        for cc in range(4):
            t = ps["sb"].tile([128, 256, 8], stag_dtype, name=f"{prefix}stag{cc}",
                              tag=f"stag{cc}", bufs=2)
            stag.append(t)
        for c2 in range(4):                 # 2-row chunks within group
            chunk = g * 4 + c2
            x_t = []
            for cc in range(4):
                t = sb.tile([128, 512], F32R, name=f"{prefix}x{cc}",
                            tag=f"x{cc}", bufs=4)
                x_load(cc, chunk, t)
                x_t.append(t)
            # --- q/k projections: out [o-chunk 128, 512 pix] ---
            q_sb, k_sb = [], []
            for wname, bname, dst in (("wq", "bq", q_sb), ("wk", "bk", k_sb)):
                for oc in range(4):
                    pp = ps["ps"].tile([128, 512], F32, name="pp", tag="pp", bufs=2)
                    for c4 in range(4):
                        nc.tensor.matmul(
                            pp[:],
                            w[wname][c4][:, oc * 128:(oc + 1) * 128],
                            x_t[c4][:],
                            start=(c4 == 0), stop=(c4 == 3),
                        )
                    t = sb.tile([128, 512], F32R, name=f"{wname}o{oc}",
                                tag=f"{wname}o", bufs=5)
                    nc.scalar.activation(
                        t[:], pp[:], mybir.ActivationFunctionType.Identity,
                        bias=w[bname][oc][:],
                    )
                    dst.append(t)
            # --- v projected transposed [pix-chunk 128, 8 heads x 64],
            #     v-bias added via a K=1 ones x bvr matmul ---
            vT_sb = []
            for pc in range(4):
                pv = ps["ps"].tile([128, 512], F32, name="pp", tag="pp", bufs=2)
                for c4 in range(4):
                    nc.tensor.matmul(
                        pv[:],
                        x_t[c4][:, pc * 128:(pc + 1) * 128],
                        w["wv"][c4][:],
                        start=(c4 == 0), stop=False,
                    )
                nc.tensor.matmul(
                    pv[:], ones_sb[0:1, :], bvr[:],
                    start=False, stop=True,
                )
                t = sb.tile([128, 512], F32R, name=f"vT{pc}", tag="vT", bufs=5)
                nc.scalar.copy(t[:], pv[:])
                vT_sb.append(t)
            # --- attention per (row-in-chunk, head) ---
            for r in range(2):
                for h in range(H):
                    m, ph = h // 2, (h % 2) * 64
                    at = ps["ps"].tile([128, 512], F32, name="at",
                                       tag="at", bufs=2)
                    for jh in range(2):
                        nc.tensor.matmul(
                            at[:, jh * 256:(jh + 1) * 256],
                            k_sb[m][ph:ph + 64,
                                    r * 256 + jh * 128:
                                    r * 256 + (jh + 1) * 128],
                            q_sb[m][ph:ph + 64, r * 256:(r + 1) * 256],
                            start=True, stop=True,
                        )
                    e_t = sb.tile([128, 512], F32R, name="e_t",
                                  tag="e_t", bufs=4)
                    nc.scalar.activation(e_t[:], at[:],
                                         mybir.ActivationFunctionType.Exp)
                    # ob: cols 0:256 = attn @ v (unnormalized), 256:512 =
                    # softmax denominators broadcast over 64 partitions
                    ob = ps["ps"].tile([64, 512], F32, name="ob", tag="ob",
                                       bufs=3)
                    for jh in range(2):
                        nc.tensor.matmul(
                            ob[:, 0:256],
                            vT_sb[2 * r + jh][:, h * 64:h * 64 + 64],
                            e_t[:, jh * 256:(jh + 1) * 256],
                            start=(jh == 0), stop=(jh == 1),
                        )
                    for jh in range(2):
                        nc.tensor.matmul(
                            ob[:, 256:512],
                            onescol[:, 0:64],
                            e_t[:, jh * 256:(jh + 1) * 256],
                            start=(jh == 0), stop=(jh == 1),
                        )
                    r_sb = sb.tile([64, 256], F32, name="r_sb", tag="r_sb",
                                   bufs=3)
                    nc.vector.reciprocal(r_sb[:], ob[:, 256:512])
                    # normalized output straight into staging (out-shift ok)
                    with nc.allow_low_precision(reason="fp32r staging"):
                        nc.vector.tensor_mul(
                            stag[m][ph:ph + 64, :, c2 * 2 + r],
                            ob[:, 0:256], r_sb[:])
            # bulk residual: stag[cc][:, :, slot0:slot0+2] += x (on gpsimd)
            for cc in range(4):
                with nc.allow_low_precision(reason="fp32r staging"):
                    nc.gpsimd.tensor_add(
                        stag[cc][:, :, c2 * 2:c2 * 2 + 2],
                        stag[cc][:, :, c2 * 2:c2 * 2 + 2],
                        x_t[cc][:].rearrange("c (r l) -> c l r", r=2))
        for cc in range(4):
            out_store(cc, g, stag[cc])


def _build(variant="full"):
    ndev = 1 if variant == "sim1" else N_CORES
    nc = bacc.Bacc("TRN2", target_bir_lowering=False, debug=False,
                   num_devices=ndev)
    if variant == "noop":
        xi = nc.dram_tensor("xi", [128, 512], F32, kind="ExternalInput").ap()
        y = nc.dram_tensor("y", [128, 512], F32, kind="ExternalOutput").ap()
        with tile.TileContext(nc) as tc:
            with tc.tile_pool(name="sb", bufs=1) as sb:
                t = sb.tile([128, 512], F32, name="t")
                nc.sync.dma_start(t[:], xi[:])
                nc.sync.dma_start(y[:], t[:])
        nc.compile()
        return nc
    if variant == "a2aonly":
        xi = nc.dram_tensor("xi", [128, 512], F32, kind="ExternalInput").ap()
        y = nc.dram_tensor("y", [128, 512], F32, kind="ExternalOutput").ap()
        with tile.TileContext(nc) as tc:
            with tc.tile_pool(name="sb", bufs=1) as sb, \
                 tc.tile_pool(name="dram", bufs=1, space="DRAM") as dram:
                a_in = dram.tile([N_CORES, D, RLOC, RLOC], F32, name="a_in")
                a_out = dram.tile([N_CORES, D, RLOC, RLOC], F32, name="a_out")
                t = sb.tile([128, 512], F32, name="t")
                nc.sync.dma_start(t[:], xi[:])
                nc.sync.dma_start(a_in[0, 0:128, 0:16, :], t[:])
                nc.gpsimd.collective_compute(
                    "AllToAll", mybir.AluOpType.bypass,
                    replica_groups=[list(range(N_CORES))],
                    ins=[a_in.opt()], outs=[a_out.opt()])
                t2 = sb.tile([128, 512], F32, name="t2")
                nc.sync.dma_start(t2[:], a_out[0, 0:128, 0:16, :])
                nc.sync.dma_start(y[:], t2[:])
        nc.compile()
        return nc
    xi = nc.dram_tensor("xi", [D, PIX], F32R, kind="ExternalInput").ap()
    y = nc.dram_tensor("y", [D, S, RLOC], F32, kind="ExternalOutput").ap()
    w_ins = {}
    for p in ("1", "2"):
        ins = []
        for nm, shp in (("wq", [D, D]), ("wk", [D, D]), ("wv", [D, D]),
                        ("bq", [D, 1]), ("bk", [D, 1]), ("bv", [H, DH, 1])):
            dt = F32R if nm.startswith("w") else F32
            ins.append(nc.dram_tensor(nm + p, shp, dt, kind="ExternalInput").ap())
        w_ins[p] = ins

    with tile.TileContext(nc) as tc:
        with tc.tile_pool(name="sb", bufs=1) as sb, \
             tc.tile_pool(name="psum", bufs=1, space="PSUM") as psp, \
             tc.tile_pool(name="dram", bufs=1, space="DRAM") as dram:
            ps = {"ps": psp, "sb": sb}
            a2a_in = dram.tile([N_CORES, D, RLOC, RLOC], F32R, name="a2a_in")
            a2a_out = dram.tile([N_CORES, D, RLOC, RLOC], F32R, name="a2a_out")

            ones_sb = sb.tile([1, 128], F32R, name="ones_sb", bufs=1)
            nc.gpsimd.memset(ones_sb[:].bitcast(mybir.dt.uint32), 0x3F800000)
            onescol = sb.tile([128, 64], F32R, name="onescol", bufs=1)
            nc.gpsimd.memset(onescol[:].bitcast(mybir.dt.uint32), 0x3F800000)
            w1 = _load_weights(nc, sb, "s1", w_ins["1"])
            w2 = _load_weights(nc, sb, "s2", w_ins["2"])

            # ---- stage 1: row attention, S-sharded ----
            def x_load1(cc, chunk, t):
                nc.sync.dma_start(
                    t[:], xi[cc * 128:(cc + 1) * 128, chunk * 512:(chunk + 1) * 512])

            def out_store1(cc, g, stg):
                # staging [128c, 256l, 8r] -> a2a_in[j, c, l32, s=g*8+r]
                for j in range(N_CORES):
                    nc.scalar.dma_start(
                        a2a_in[j, cc * 128:(cc + 1) * 128, :, g * 8:(g + 1) * 8],
                        stg[:, j * 32:(j + 1) * 32, :])

            _stage(tc, nc, sb, ps, w1, ones_sb, onescol, w1["bvr"], x_load1,
                   out_store1, "s1", stag_dtype=F32R)

            if variant in ("noa2a", "sim1"):
                # timing-only variant: local copy instead of the collective
                for j in range(N_CORES):
                    nc.gpsimd.dma_start(a2a_out[j], a2a_in[j])
            else:
                nc.gpsimd.collective_compute(
                    "AllToAll", mybir.AluOpType.bypass,
                    replica_groups=[list(range(N_CORES))],
                    ins=[a2a_in.opt()], outs=[a2a_out.opt()],
                )

            # ---- stage 2: column attention, L-sharded ----
            def x_load2(cc, chunk, t):
                for lr in range(2):
                    src = a2a_out[:, cc * 128:(cc + 1) * 128, chunk * 2 + lr, :]
                    nc.sync.dma_start(
                        t[:, lr * 256:(lr + 1) * 256].rearrange(
                            "c (j s) -> c j s", j=8),
                        src.rearrange("j c s -> c j s"))

            def out_store2(cc, g, stg):
                # staging [128c, 256s, 8l] -> y[c, s, l=g*8..]
                nc.scalar.dma_start(y[cc * 128:(cc + 1) * 128, :, g * 8:(g + 1) * 8],
                                    stg[:])

            _stage(tc, nc, sb, ps, w2, ones_sb, onescol, w2["bvr"], x_load2,
                   out_store2, "s2")

    nc.compile()
    return nc


def _get_nc(variant="full"):
    key = "nc:" + variant
    if key not in _CACHE:
        _CACHE[key] = _build(variant)
    return _CACHE[key]


def _in_maps(x, Wr, br, Wc, bc):
    x = np.asarray(x, dtype=np.float32)
    stage_w = {}
    for p, W, b in (("1", np.asarray(Wr, np.float32), np.asarray(br, np.float32)),
                    ("2", np.asarray(Wc, np.float32), np.asarray(bc, np.float32))):
        stage_w["wq" + p] = np.ascontiguousarray(W[0:D].T)
        stage_w["wk" + p] = np.ascontiguousarray(W[D:2 * D].T)
        stage_w["wv" + p] = np.ascontiguousarray(W[2 * D:3 * D].T)
        stage_w["bq" + p] = np.ascontiguousarray(b[0:D].reshape(D, 1))
        stage_w["bk" + p] = np.ascontiguousarray(b[D:2 * D].reshape(D, 1))
        stage_w["bv" + p] = np.ascontiguousarray(b[2 * D:3 * D].reshape(H, DH, 1))
    maps = []
    for i in range(N_CORES):
        m = {"xi": np.ascontiguousarray(
            x[0, :, i * RLOC:(i + 1) * RLOC, :].reshape(D, PIX))}
        m.update(stage_w)
        maps.append(m)
    return maps


def _get_runner(variant="full"):
    """Build (once) a cached jitted shard_map callable over the 8 cores."""
    rkey = "runner:" + variant
    if rkey in _CACHE:
        return _CACHE[rkey]
    import jax
    from jax.sharding import Mesh, PartitionSpec
    from jax.experimental.shard_map import shard_map
    from concourse import bass2jax as b2j

    nc = _get_nc(variant)
    b2j.install_neuronx_cc_hook()
    part_name = nc.partition_id_tensor.name if nc.partition_id_tensor else None
    in_names, out_names, out_avals, zero_outs = [], [], [], []
    for alloc in nc.m.functions[0].allocations:
        if not isinstance(alloc, mybir.MemoryLocationSet):
            continue
        name = alloc.memorylocations[0].name
        if alloc.kind == "ExternalInput":
            if name != part_name:
                in_names.append(name)
        elif alloc.kind == "ExternalOutput":
            out_names.append(name)
            shape = tuple(alloc.tensor_shape)
            dtype = mybir.dt.np(alloc.dtype)
            out_avals.append(jax.core.ShapedArray(shape, dtype))
            zero_outs.append(np.zeros(shape, dtype))
    n_params = len(in_names)
    all_names = in_names + out_names
    if part_name is not None:
        all_names = all_names + [part_name]

    def _body(*args):
        operands = list(args)
        if part_name is not None:
            operands.append(b2j.partition_id_tensor())
        outs = b2j._bass_exec_p.bind(
            *operands,
            out_avals=tuple(out_avals),
            in_names=tuple(all_names),
            out_names=tuple(out_names),
            lowering_input_output_aliases=(),
            sim_require_finite=True,
            sim_require_nnan=True,
            nc=nc,
        )
        return tuple(outs)

    devices = jax.devices()[:N_CORES]
    mesh = Mesh(np.asarray(devices), ("core",))
    specs = (PartitionSpec("core"),) * (n_params + len(out_names))
    sharded = jax.jit(
        shard_map(_body, mesh=mesh, in_specs=specs,
                  out_specs=(PartitionSpec("core"),) * len(out_names),
                  check_rep=False),
        keep_unused=True,
    )
    concat_zeros = [
        jax.device_put(
            np.zeros((N_CORES * z.shape[0], *z.shape[1:]), z.dtype),
            jax.sharding.NamedSharding(mesh, PartitionSpec("core")))
        for z in zero_outs
    ]
    _CACHE[rkey] = (sharded, in_names, out_names, out_avals, concat_zeros)
    return _CACHE[rkey]


def _run(maps):
    sharded, in_names, out_names, out_avals, concat_zeros = _get_runner()
    concat_in = [
        np.concatenate([maps[c][nm] for c in range(N_CORES)], axis=0)
        for nm in in_names
    ]
    out_arrs = sharded(*concat_in, *concat_zeros)
    return [
        {nm: np.asarray(out_arrs[i]).reshape(N_CORES, *out_avals[i].shape)[c]
         for i, nm in enumerate(out_names)}
        for c in range(N_CORES)
    ]


def kernel(x, Wr, br, Wc, bc):
    maps = _in_maps(x, Wr, br, Wc, bc)
    results = _run(maps)
    out = np.concatenate([results[i]["y"] for i in range(N_CORES)], axis=2)
    return out[None].astype(np.float32)
